# revision 8
# baseline (speedup 1.0000x reference)
# Trainium2 Bass kernel for fixed-size radius_graph (per-node K nearest
# neighbors within a cutoff, batched over independent graphs).
#
# Problem geometry (hardcoded per spec): B=1024 graphs x M=256 atoms, K=32
# neighbors, cutoff^2 = 25.0. batch == repeat(arange(B), M).
#
# Sharding: graphs are data-parallel across the 8 NeuronCores (128 graphs
# per core); no cross-core communication.
#
# Device (per graph): key[i,j] = -d2[i,j] = 2*dot(p_i,p_j) - |p_i|^2 - |p_j|^2
# via a 14-row fp32 PE matmul in which every coordinate is pre-split into a
# bf16-exact high part plus a 16-bit-mantissa low part. The PE's fp32 mode
# drops the lo*lo cross term of its internal bf16 split, so feeding it
# operands whose internal split is lossless makes every product exact and
# keeps d2 at true-f32 accuracy (the naive 5-row form loses ~3e-2 absolute).
# The diagonal (self-pair) is pushed to -3e9 with a constant tile, then 4
# rounds of (max8, max_index8, match_replace8) on the vector engine extract
# the 32 smallest d2 per row in sorted order with jax.lax.top_k tie
# semantics. Weights sqrt on the scalar engine; masks and index arithmetic
# on gpsimd.
#
# Host: operand prep, plus assembly of edge_index row 1 (centers) and
# edge_vec = pos[nbr] - pos[ctr] (pure indexing of the inputs by the
# device-computed neighbor list; the platform's indirect-gather DMA path is
# broken/slow, see dev notes).

import numpy as np

B = 1024
M = 256
K = 32
N_CORES = 8
G = B // N_CORES          # graphs per core
KROWS = 14                # contraction rows of the split matmul
CUT2 = 25.0
DIAG_VAL = 3.0e9          # pushes self-pair below every real candidate
SENT = -4.0e9             # match_replace sentinel, below the diagonal keys

_PROGRAM_CACHE = {}

# Dev toggles (test.py sets these; harness uses defaults).
TRACE = False
LAST_RESULTS = None


def _build_program(g_count, unroll=4):
    import concourse.bass as bass
    import concourse.mybir as mybir
    import concourse.tile as tile
    from concourse import bacc

    f32 = mybir.dt.float32
    i32 = mybir.dt.int32
    u32 = mybir.dt.uint32
    Alu = mybir.AluOpType

    nc = bacc.Bacc("TRN2", target_bir_lowering=False, debug=False)

    lhsT_all = nc.dram_tensor(
        "lhsT_all", [g_count * KROWS, M], f32, kind="ExternalInput"
    )
    rhs_all = nc.dram_tensor(
        "rhs_all", [g_count * KROWS, M], f32, kind="ExternalInput"
    )
    gbase_all = nc.dram_tensor("gbase_all", [g_count * 128], f32, kind="ExternalInput")
    diag_in = nc.dram_tensor("diag_in", [2 * 128, M], f32, kind="ExternalInput")

    o_nbr = nc.dram_tensor("o_nbr", [g_count * M, K], i32, kind="ExternalOutput")
    o_w = nc.dram_tensor("o_w", [g_count * M, K], f32, kind="ExternalOutput")

    lhsT_ap = lhsT_all.ap()
    rhs_ap = rhs_all.ap()
    gbase_ap = gbase_all.ap()
    diag_ap = diag_in.ap()
    o_nbr_ap = o_nbr.ap()
    o_w_ap = o_w.ap()

    ds = bass.ds

    with tile.TileContext(nc) as tc:
        with (
            tc.tile_pool(name="const", bufs=1) as cp,
            tc.tile_pool(name="gops", bufs=3) as gp,
            tc.tile_pool(name="keys", bufs=3) as kp,
            tc.tile_pool(name="small", bufs=4) as sp,
            tc.tile_pool(name="psum", bufs=4, space="PSUM") as pp,
        ):
            diag_t = cp.tile([128, 2 * M], f32, tag="diag")
            nc.sync.dma_start(out=diag_t[:, 0:M], in_=diag_ap[0:128, :])
            nc.sync.dma_start(out=diag_t[:, M : 2 * M], in_=diag_ap[128:, :])

            def block_body(g, bi):
                row0 = g * M + bi * 128

                lhsT_t = gp.tile([KROWS, 128], f32, tag="lhsT")
                nc.sync.dma_start(
                    out=lhsT_t[:, :],
                    in_=lhsT_ap[ds(g * KROWS, KROWS), ds(bi * 128, 128)],
                )
                rhs_t = gp.tile([KROWS, M], f32, tag="rhs")
                nc.sync.dma_start(out=rhs_t[:, :], in_=rhs_ap[ds(g * KROWS, KROWS), :])

                psum_t = pp.tile([128, M], f32)
                nc.tensor.matmul(
                    out=psum_t[:, :],
                    lhsT=lhsT_t[:, :],
                    rhs=rhs_t[:, :],
                    start=True,
                    stop=True,
                )

                key = kp.tile([128, M], f32, tag="key")
                nc.vector.tensor_tensor(
                    out=key[:, :],
                    in0=psum_t[:, :],
                    in1=diag_t[:, bi * M : (bi + 1) * M],
                    op=Alu.subtract,
                )

                kmax = sp.tile([128, K], f32, tag="kmax")
                kidx = sp.tile([128, K], u32, tag="kidx")
                for r in range(4):
                    sl = slice(8 * r, 8 * r + 8)
                    nc.vector.max(out=kmax[:, sl], in_=key[:, :])
                    nc.vector.max_index(
                        out=kidx[:, sl], in_max=kmax[:, sl], in_values=key[:, :]
                    )
                    if r < 3:
                        nc.vector.match_replace(
                            out=key[:, :],
                            in_to_replace=kmax[:, sl],
                            in_values=key[:, :],
                            imm_value=SENT,
                        )

                gbase_t = sp.tile([128, 1], f32, tag="gbase")
                nc.sync.dma_start(
                    out=gbase_t[:, :], in_=gbase_ap[ds(g * 128, 128), None]
                )

                validf = sp.tile([128, K], f32, tag="validf")
                nc.gpsimd.tensor_scalar(validf[:, :], kmax[:, :], -CUT2, None, Alu.is_gt)
                invf = sp.tile([128, K], f32, tag="invf")
                nc.gpsimd.tensor_scalar(invf[:, :], kmax[:, :], -CUT2, None, Alu.is_le)

                # w = sqrt(max(-kmax, 0)) * validf
                d2c = sp.tile([128, K], f32, tag="d2c")
                nc.gpsimd.tensor_scalar(
                    d2c[:, :], kmax[:, :], -1.0, 0.0, Alu.mult, Alu.max
                )
                wraw = sp.tile([128, K], f32, tag="wraw")
                nc.scalar.activation(
                    wraw[:, :], d2c[:, :], mybir.ActivationFunctionType.Sqrt
                )
                w_t = sp.tile([128, K], f32, tag="w")
                nc.gpsimd.tensor_tensor(
                    w_t[:, :], wraw[:, :], validf[:, :], op=Alu.mult
                )

                # nbr = valid ? (kidx + g*M) : -1   (f32 math, exact below 2^24)
                kidx_f = sp.tile([128, K], f32, tag="kidxf")
                nc.gpsimd.tensor_copy(out=kidx_f[:, :], in_=kidx[:, :])
                globn = sp.tile([128, K], f32, tag="globn")
                nc.gpsimd.tensor_scalar(
                    globn[:, :], kidx_f[:, :], gbase_t[:, :], None, Alu.add
                )
                tg = sp.tile([128, K], f32, tag="tg")
                nc.gpsimd.tensor_tensor(
                    tg[:, :], globn[:, :], validf[:, :], op=Alu.mult
                )
                nbr_f = sp.tile([128, K], f32, tag="nbrf")
                nc.gpsimd.tensor_tensor(
                    nbr_f[:, :], tg[:, :], invf[:, :], op=Alu.subtract
                )
                nbr_o = sp.tile([128, K], i32, tag="nbro")
                nc.gpsimd.tensor_copy(out=nbr_o[:, :], in_=nbr_f[:, :])

                nc.scalar.dma_start(out=o_w_ap[ds(row0, 128), :], in_=w_t[:, :])
                nc.scalar.dma_start(out=o_nbr_ap[ds(row0, 128), :], in_=nbr_o[:, :])

            def graph_body(g):
                block_body(g, 0)
                block_body(g, 1)

            if g_count % unroll == 0 and g_count > unroll:
                with tc.For_i(0, g_count, unroll) as gv:
                    for u in range(unroll):
                        graph_body(gv + u)
            else:
                for g in range(g_count):
                    graph_body(g)

    nc.compile()
    return nc


def _split_bf16(a):
    import ml_dtypes

    hi = a.astype(ml_dtypes.bfloat16).astype(np.float32)
    lo = (a - hi).astype(np.float32)
    return hi, lo


def _host_prep(pos, g_count=G, n_cores=N_CORES):
    """Build per-core input maps. pos: [n_cores*g_count*M, 3] float32."""
    p = pos.reshape(-1, M, 3)
    f32 = np.float32
    x, y, z = p[..., 0], p[..., 1], p[..., 2]
    sq = ((x * x) + (y * y)) + (z * z)  # f32 stepwise, matches jnp.sum order
    sq = sq.astype(f32)
    ones = np.ones_like(sq)

    rows_l = []
    rows_r = []
    for c in (x, y, z):
        ch, cl = _split_bf16(c)
        # lhsT k-rows paired with rhs k-rows:
        #   2ch*ch', 2ch*cl', 2cl*ch', 2cl*cl'
        rows_l += [2.0 * ch, 2.0 * ch, 2.0 * cl, 2.0 * cl]
        rows_r += [ch, cl, ch, cl]
    rows_l += [-sq, -ones]
    rows_r += [ones, sq]
    lhsT = np.stack(rows_l, axis=1).astype(f32)  # [B, 14, M]
    rhs = np.stack(rows_r, axis=1).astype(f32)

    gbase = np.repeat(np.arange(g_count, dtype=np.int32) * M, 128).astype(f32)
    diag = np.zeros((2 * 128, M), dtype=f32)
    for b in range(2):
        diag[b * 128 + np.arange(128), b * 128 + np.arange(128)] = DIAG_VAL

    in_maps = []
    for c in range(n_cores):
        gs = slice(c * g_count, (c + 1) * g_count)
        in_maps.append(
            {
                "lhsT_all": lhsT[gs].reshape(g_count * KROWS, M).copy(),
                "rhs_all": rhs[gs].reshape(g_count * KROWS, M).copy(),
                "gbase_all": gbase.copy(),
                "diag_in": diag.copy(),
            }
        )
    return in_maps


def _assemble(results, pos, g_count=G, n_cores=N_CORES):
    nbrs, ws = [], []
    for c, res in enumerate(results):
        base = np.int32(c * g_count * M)
        nbr = res["o_nbr"].reshape(-1)
        nbrs.append(np.where(nbr >= 0, nbr + base, np.int32(-1)))
        ws.append(res["o_w"].reshape(-1))
    nbr = np.concatenate(nbrs)
    w = np.concatenate(ws).astype(np.float32)
    valid = nbr >= 0
    ctr = np.where(
        valid, np.repeat(np.arange(B * M, dtype=np.int32), K), np.int32(-1)
    )
    edge_index = np.stack([nbr, ctr]).astype(np.int32)
    # edge_vec by direct indexing of the input positions (0 where invalid)
    nbr_safe = np.where(valid, nbr, 0)
    ctr_safe = np.where(valid, ctr, 0)
    edge_vec = np.where(
        valid[:, None], pos[nbr_safe] - pos[ctr_safe], np.float32(0.0)
    ).astype(np.float32)
    return edge_index, w, edge_vec


def kernel(pos, batch):
    from concourse.bass_utils import run_bass_kernel_spmd

    pos = np.ascontiguousarray(np.asarray(pos, dtype=np.float32))
    assert pos.shape == (B * M, 3)

    key = ("prog", G)
    if key not in _PROGRAM_CACHE:
        _PROGRAM_CACHE[key] = _build_program(G)
    nc = _PROGRAM_CACHE[key]

    in_maps = _host_prep(pos)
    res = run_bass_kernel_spmd(
        nc, in_maps, core_ids=list(range(N_CORES)), trace=TRACE
    )
    global LAST_RESULTS
    LAST_RESULTS = res
    return _assemble(res.results, pos)


# revision 23
# speedup vs baseline: 1.0151x; 1.0151x over previous
# Trainium2 Bass kernel for fixed-size radius_graph (per-node K nearest
# neighbors within a cutoff, batched over independent graphs).
#
# Problem geometry (hardcoded per spec): B=1024 graphs x M=256 atoms, K=32
# neighbors, cutoff^2 = 25.0. batch == repeat(arange(B), M).
#
# Sharding: graphs are data-parallel across the 8 NeuronCores (128 graphs
# per core); no cross-core communication.
#
# Device (per graph): key[i,j] = -d2[i,j] = 2*dot(p_i,p_j) - |p_i|^2 - |p_j|^2
# via a 14-row fp32 PE matmul in which every coordinate is pre-split into a
# bf16-exact high part plus a 16-bit-mantissa low part. The PE's fp32 mode
# drops the lo*lo cross term of its internal bf16 split, so feeding it
# operands whose internal split is lossless makes every product exact and
# keeps d2 at true-f32 accuracy (the naive 5-row form loses ~3e-2 absolute).
# The diagonal (self-pair) is pushed to -3e9 with a constant tile, then 4
# rounds of (max8, max_index8, match_replace8) on the vector engine extract
# the 32 smallest d2 per row in sorted order with jax.lax.top_k tie
# semantics. Weights sqrt on the scalar engine; masks and index arithmetic
# on gpsimd.
#
# Host: operand prep, plus assembly of edge_index row 1 (centers) and
# edge_vec = pos[nbr] - pos[ctr] (pure indexing of the inputs by the
# device-computed neighbor list; the platform's indirect-gather DMA path is
# broken/slow, see dev notes).

import numpy as np

B = 1024
M = 256
K = 32
N_CORES = 8
G = B // N_CORES          # graphs per core
KROWS = 14                # contraction rows of the split matmul
CUT2 = 25.0
DIAG_VAL = 3.0e9          # pushes self-pair below every real candidate
SENT = -4.0e9             # match_replace sentinel, below the diagonal keys

_PROGRAM_CACHE = {}

# Dev toggles (test.py sets these; harness uses defaults).
TRACE = False
LAST_RESULTS = None


def _build_program(g_count, unroll=4, body_repeat=1, ablate=None, smalls="pool"):
    import concourse.bass as bass
    import concourse.mybir as mybir
    import concourse.tile as tile
    from concourse import bacc

    f32 = mybir.dt.float32
    i32 = mybir.dt.int32
    u32 = mybir.dt.uint32
    Alu = mybir.AluOpType

    nc = bacc.Bacc("TRN2", target_bir_lowering=False, debug=False)

    lhsT_all = nc.dram_tensor(
        "lhsT_all", [g_count * KROWS, M], f32, kind="ExternalInput"
    )
    rhs_all = nc.dram_tensor(
        "rhs_all", [g_count * KROWS, M], f32, kind="ExternalInput"
    )
    gbase_all = nc.dram_tensor("gbase_all", [128, g_count], f32, kind="ExternalInput")
    diag_in = nc.dram_tensor("diag_in", [2 * 128, M], f32, kind="ExternalInput")

    o_nbr = nc.dram_tensor("o_nbr", [g_count * M, K], i32, kind="ExternalOutput")
    o_w = nc.dram_tensor("o_w", [g_count * M, K], f32, kind="ExternalOutput")

    lhsT_ap = lhsT_all.ap()
    rhs_ap = rhs_all.ap()
    gbase_ap = gbase_all.ap()
    diag_ap = diag_in.ap()
    o_nbr_ap = o_nbr.ap()
    o_w_ap = o_w.ap()

    ds = bass.ds

    with tile.TileContext(nc) as tc:
        with (
            tc.tile_pool(name="const", bufs=1) as cp,
            tc.tile_pool(name="gops", bufs=3) as gp,
            tc.tile_pool(name="keys", bufs=3) as kp,
            tc.tile_pool(name="small", bufs=4) as sp,
            tc.tile_pool(name="psum", bufs=4, space="PSUM") as pp,
        ):
            diag_t = cp.tile([128, 2 * M], f32, tag="diag")
            nc.sync.dma_start(out=diag_t[:, 0:M], in_=diag_ap[0:128, :])
            nc.sync.dma_start(out=diag_t[:, M : 2 * M], in_=diag_ap[128:, :])
            gbase_t = cp.tile([128, g_count], f32, tag="gbase")
            nc.sync.dma_start(out=gbase_t[:, :], in_=gbase_ap[:, :])

            def load_graph(g):
                lhsT_t = gp.tile([KROWS, M], f32, tag="lhsT")
                nc.sync.dma_start(
                    out=lhsT_t[:, :], in_=lhsT_ap[ds(g * KROWS, KROWS), :]
                )
                rhs_t = gp.tile([KROWS, M], f32, tag="rhs")
                nc.sync.dma_start(out=rhs_t[:, :], in_=rhs_ap[ds(g * KROWS, KROWS), :])
                return lhsT_t, rhs_t

            def block_body(g, bi, lhsT_t, rhs_t):
                row0 = g * M + bi * 128

                psum_t = pp.tile([128, M], f32)
                nc.tensor.matmul(
                    out=psum_t[:, :],
                    lhsT=lhsT_t[:, bi * 128 : (bi + 1) * 128],
                    rhs=rhs_t[:, :],
                    start=True,
                    stop=True,
                )

                key = kp.tile([128, M], f32, tag="key")
                nc.vector.tensor_tensor(
                    out=key[:, :],
                    in0=psum_t[:, :],
                    in1=diag_t[:, bi * M : (bi + 1) * M],
                    op=Alu.subtract,
                )

                kmax = sp.tile([128, K], f32, tag="kmax")
                kidx = sp.tile([128, K], u32, tag="kidx")
                if ablate in ("nomi", "justsel"):
                    nc.gpsimd.memset(kidx[:, :], 0)
                for r in range(4):
                    sl = slice(8 * r, 8 * r + 8)
                    nc.vector.max(out=kmax[:, sl], in_=key[:, :])
                    if ablate not in ("nomi", "justsel"):
                        nc.vector.max_index(
                            out=kidx[:, sl], in_max=kmax[:, sl], in_values=key[:, :]
                        )
                    if r < 3 and ablate != "nomr":
                        nc.vector.match_replace(
                            out=key[:, :],
                            in_to_replace=kmax[:, sl],
                            in_values=key[:, :],
                            imm_value=SENT,
                        )

                if ablate == "justsel":
                    nc.scalar.dma_start(out=o_w_ap[ds(row0, 128), :], in_=kmax[:, :])
                    nc.scalar.dma_start(
                        out=o_nbr_ap[ds(row0, 128), :], in_=kidx[:, :].bitcast(i32)
                    )
                    return

                gb = gbase_t[:, ds(g, 1)]
                se = nc.gpsimd if smalls == "pool" else nc.vector

                validf = sp.tile([128, K], f32, tag="validf")
                se.tensor_scalar(validf[:, :], kmax[:, :], -CUT2, None, Alu.is_gt)
                invf = sp.tile([128, K], f32, tag="invf")
                se.tensor_scalar(invf[:, :], kmax[:, :], -CUT2, None, Alu.is_le)

                # w = sqrt(max(-kmax, 0)) * validf
                d2c = sp.tile([128, K], f32, tag="d2c")
                se.tensor_scalar(
                    d2c[:, :], kmax[:, :], -1.0, 0.0, Alu.mult, Alu.max
                )
                wraw = sp.tile([128, K], f32, tag="wraw")
                nc.scalar.activation(
                    wraw[:, :], d2c[:, :], mybir.ActivationFunctionType.Sqrt
                )
                w_t = sp.tile([128, K], f32, tag="w")
                se.tensor_tensor(
                    w_t[:, :], wraw[:, :], validf[:, :], op=Alu.mult
                )

                # nbr = valid ? (kidx + g*M) : -1   (f32 math, exact below 2^24)
                kidx_f = sp.tile([128, K], f32, tag="kidxf")
                se.tensor_copy(out=kidx_f[:, :], in_=kidx[:, :])
                tg = sp.tile([128, K], f32, tag="tg")
                if smalls == "pool":
                    globn = sp.tile([128, K], f32, tag="globn")
                    se.tensor_scalar(globn[:, :], kidx_f[:, :], gb, None, Alu.add)
                    se.tensor_tensor(
                        tg[:, :], globn[:, :], validf[:, :], op=Alu.mult
                    )
                else:
                    se.scalar_tensor_tensor(
                        out=tg[:, :],
                        in0=kidx_f[:, :],
                        scalar=gb,
                        in1=validf[:, :],
                        op0=Alu.add,
                        op1=Alu.mult,
                    )
                nbr_f = sp.tile([128, K], f32, tag="nbrf")
                se.tensor_tensor(
                    nbr_f[:, :], tg[:, :], invf[:, :], op=Alu.subtract
                )
                nbr_o = sp.tile([128, K], i32, tag="nbro")
                se.tensor_copy(out=nbr_o[:, :], in_=nbr_f[:, :])

                nc.scalar.dma_start(out=o_w_ap[ds(row0, 128), :], in_=w_t[:, :])
                nc.scalar.dma_start(out=o_nbr_ap[ds(row0, 128), :], in_=nbr_o[:, :])

            def graph_body(g):
                for _ in range(body_repeat):
                    lhsT_t, rhs_t = load_graph(g)
                    block_body(g, 0, lhsT_t, rhs_t)
                    block_body(g, 1, lhsT_t, rhs_t)

            if g_count % unroll == 0 and g_count > unroll:
                with tc.For_i(0, g_count, unroll) as gv:
                    for u in range(unroll):
                        graph_body(gv + u)
            else:
                for g in range(g_count):
                    graph_body(g)

    nc.compile()
    return nc


def _split_bf16(a):
    import ml_dtypes

    hi = a.astype(ml_dtypes.bfloat16).astype(np.float32)
    lo = (a - hi).astype(np.float32)
    return hi, lo


def _host_prep(pos, g_count=G, n_cores=N_CORES):
    """Build per-core input maps. pos: [n_cores*g_count*M, 3] float32."""
    p = pos.reshape(-1, M, 3)
    f32 = np.float32
    x, y, z = p[..., 0], p[..., 1], p[..., 2]
    sq = ((x * x) + (y * y)) + (z * z)  # f32 stepwise, matches jnp.sum order
    sq = sq.astype(f32)
    ones = np.ones_like(sq)

    rows_l = []
    rows_r = []
    for c in (x, y, z):
        ch, cl = _split_bf16(c)
        # lhsT k-rows paired with rhs k-rows:
        #   2ch*ch', 2ch*cl', 2cl*ch', 2cl*cl'
        rows_l += [2.0 * ch, 2.0 * ch, 2.0 * cl, 2.0 * cl]
        rows_r += [ch, cl, ch, cl]
    rows_l += [-sq, -ones]
    rows_r += [ones, sq]
    lhsT = np.stack(rows_l, axis=1).astype(f32)  # [B, 14, M]
    rhs = np.stack(rows_r, axis=1).astype(f32)

    gbase = np.broadcast_to(
        (np.arange(g_count, dtype=np.int32) * M).astype(f32)[None, :], (128, g_count)
    ).copy()
    diag = np.zeros((2 * 128, M), dtype=f32)
    for b in range(2):
        diag[b * 128 + np.arange(128), b * 128 + np.arange(128)] = DIAG_VAL

    in_maps = []
    for c in range(n_cores):
        gs = slice(c * g_count, (c + 1) * g_count)
        in_maps.append(
            {
                "lhsT_all": lhsT[gs].reshape(g_count * KROWS, M).copy(),
                "rhs_all": rhs[gs].reshape(g_count * KROWS, M).copy(),
                "gbase_all": gbase.copy(),
                "diag_in": diag.copy(),
            }
        )
    return in_maps


def _assemble(results, pos, g_count=G, n_cores=N_CORES):
    nbrs, ws = [], []
    for c, res in enumerate(results):
        base = np.int32(c * g_count * M)
        nbr = res["o_nbr"].reshape(-1)
        nbrs.append(np.where(nbr >= 0, nbr + base, np.int32(-1)))
        ws.append(res["o_w"].reshape(-1))
    nbr = np.concatenate(nbrs)
    w = np.concatenate(ws).astype(np.float32)
    valid = nbr >= 0
    ctr = np.where(
        valid, np.repeat(np.arange(B * M, dtype=np.int32), K), np.int32(-1)
    )
    edge_index = np.stack([nbr, ctr]).astype(np.int32)
    # edge_vec by direct indexing of the input positions (0 where invalid)
    nbr_safe = np.where(valid, nbr, 0)
    ctr_safe = np.where(valid, ctr, 0)
    edge_vec = np.where(
        valid[:, None], pos[nbr_safe] - pos[ctr_safe], np.float32(0.0)
    ).astype(np.float32)
    return edge_index, w, edge_vec


def kernel(pos, batch):
    from concourse.bass_utils import run_bass_kernel_spmd

    pos = np.ascontiguousarray(np.asarray(pos, dtype=np.float32))
    assert pos.shape == (B * M, 3)

    key = ("prog", G)
    if key not in _PROGRAM_CACHE:
        _PROGRAM_CACHE[key] = _build_program(G, unroll=8, smalls="vector")
    nc = _PROGRAM_CACHE[key]

    in_maps = _host_prep(pos)
    res = run_bass_kernel_spmd(
        nc, in_maps, core_ids=list(range(N_CORES)), trace=TRACE
    )
    global LAST_RESULTS
    LAST_RESULTS = res
    return _assemble(res.results, pos)
